# revision 36
# baseline (speedup 1.0000x reference)
"""Trainium2 Bass kernel for nn_BmmEnsemble (ANI-style per-species ensemble MLP).

Math (see reference): for each species s (4) and ensemble member e (8), the
species' atoms' AEV rows go through a 384->160->128->96->1 MLP with CELU(0.1)
after the first three layers; the output energy is the global sum over all
atoms of the ensemble-mean of the final scalar, i.e.

    E = (1/8) * sum_{s,e,n} ( g2[s,e,n,:] @ w3[s,e] + b3[s,e] )

Distribution: data-parallel over atoms. The host gathers each species' atom
rows (aev_flat[idx]) and hands each of the 8 cores a 2048-atoms-per-species
slice, pre-transposed to feature-major [384, 2048] so activations stay
feature-major on-chip (no transposes between layers). Per-species ensemble
weights are replicated to all cores. Each core returns per-(s,e) row-sums of
the last hidden layer [96, 32*4]; the host applies the tiny w3 dot, the b3
term, the ensemble mean, and the cross-core sum (the "all-reduce").

On-chip design (all matmuls float32r = full-rate rounded fp32, feature-major
layout so no transposes are ever needed between layers; all activation tiles
are full 128-partition x 512-atom tiles):
  - Layer 0 per member: psum_l0a[128,512] += w0a[k].T @ xT[k] (3 K-tiles).
  - Layer 0 "tail" (features 128:160): the four members of a quad share rhs x,
    so their 32-feature blocks are column-packed into ONE M=128 matmul per
    K-tile, landing at partition offsets 0/32/64/96 of a shared psum bank.
  - Layer 1 per member: K=128 matmul on g0a plus a K=128 matmul against the
    full merged g0b with a zero-padded w1b (member's 32 rows at offset
    32*(e%4)) - zero rows contribute nothing, and K=128 costs the same as
    K=32 (matmul time is set by the moving dim).
  - Layer 2: the quad's 4x96 outputs are packed into 3 psum banks
    (bank0 = e0[0:96]|e1[0:32], bank1 = e1[32:96]|e2[0:64],
    bank2 = e2[64:96]|e3[0:96]) via 2 zero-padded matmuls per bank.
  - ScalarE: ONE Exp pass per psum tile: e = Exp((z_pre + b)/alpha).
  - VectorE: ONE fused custom-DVE pass per psum tile:
      g = relu(z_pre + b) + min(alpha*e - alpha, 0)  ==  celu(z, alpha) exact
    (for layer 2 the same op also emits accum_out = row-sums of g2).

CELU identity used: celu(z) = relu(z) + min(alpha*(exp(z/alpha) - 1), 0).

Measured on 8 axon-tunneled trn2 cores: ~292 us HW exec, rel err 1.6e-4
(f32r matmul rounding; fp32 everywhere else). Engines all ~85-90% busy
(VectorE 90%, ScalarE 87%, TensorE 84%) - the kernel is at the balanced
three-engine floor of this dataflow: the two PSUM-read elementwise passes
(exp on ScalarE, celu-blend on VectorE) stream at 1 elem/lane/cycle and
bound the kernel at ~205-270 us/core alongside 928 matmuls at ~253 us.
"""

import os
from operator import add as _operator_add

import numpy as np

import concourse.dve_ops as _dve_ops
import concourse.mybir as mybir
import concourse.tile as tile
from concourse import bacc
from concourse.bass_utils import run_bass_kernel_spmd
from concourse.dve_spec import (
    C0,
    C1,
    Spec,
    Src0,
    Src1,
    Zero,
    _has_src1,
    lower,
    minn,
    relu,
)
from concourse.dve_uop import DveOpSpec

# ---------------------------------------------------------------- constants
S, E = 4, 8
N_ATOMS = 65536
N_CORES = 8
A_SP = N_ATOMS // S // N_CORES      # atoms per species per core = 2048
CHUNK = 512
NCH = A_SP // CHUNK                 # 4 chunks
K0, H0, H1, H2 = 384, 160, 128, 96
KT = K0 // 128                      # 3 K-tiles for layer 0
NQ = 2                              # member quads per species (E/4)
ALPHA = 0.1

F32 = mybir.dt.float32
F32R = mybir.dt.float32r
EXP = mybir.ActivationFunctionType.Exp

# ------------------------------------------------------- custom DVE ops
_BODY = relu(Src0 + C0) + minn(Src1 * C1 - C1, Zero)


def _celu_np(in0, in1, s0, s1):
    z = in0.astype(np.float32) + s0
    neg = np.minimum(in1.astype(np.float32) * s1 - s1, 0.0)
    return (np.maximum(z, 0.0) + neg).astype(np.float32)


def _ref_plain(in0, in1, s0, s1, imm2):
    return _celu_np(in0, in1, s0, s1)


def _ref_acc(in0, in1, s0, s1, imm2):
    b = _celu_np(in0, in1, s0, s1)
    return b, b.reshape(b.shape[0], -1).sum(axis=-1, keepdims=True)


def _mk_op(name, spec):
    row = _dve_ops._CUSTOM_DVE_ROW_BASE + len(_dve_ops.OPS)
    assert row < 0x20, "custom-DVE opcode rows exhausted"
    _dve_ops._SUB_OPCODE_FOR_NAME[name] = row
    shas = {}
    for ver in ("v3", "v4"):
        s = DveOpSpec(
            name=name, opcode=row, uops=lower(spec, ver=ver), rd1_en=_has_src1(spec)
        )
        shas[ver] = s.sha(ver)
    op = _dve_ops.DveOp(name, spec, subdim=False, uops_sha=shas)
    _dve_ops.OPS.append(op)
    _dve_ops.CUSTOM_DVE_SPECS[name] = spec
    return op


def _register_celu_ops():
    existing = {o.name: o for o in _dve_ops.OPS}
    if "CELU_BLEND_ANT" in existing:
        return existing["CELU_BLEND_ANT"], existing["CELU_BLEND_ACC_ANT"]
    celu = _mk_op("CELU_BLEND_ANT", Spec(body=_BODY, reference=_ref_plain))
    celu_acc = _mk_op(
        "CELU_BLEND_ACC_ANT",
        Spec(body=_BODY, accum=_operator_add, accum_init=Zero, reference=_ref_acc),
    )
    return celu, celu_acc


# ------------------------------------------------------------ device build
_NC = None


def _build_nc():
    global _NC
    if _NC is not None:
        return _NC
    CELU, CELU_ACC = _register_celu_ops()

    nc = bacc.Bacc("TRN2", target_bir_lowering=False, debug=False)

    # per-core inputs
    xt_d = nc.dram_tensor("xt", [S, KT, 128, A_SP], F32R, kind="ExternalInput")
    # replicated weight packs.
    # w0a: first 128 output features per member.  w0b4: the last 32 features
    # of FOUR members column-packed into one [*,128] stationary tile (all
    # members of a species share the same x, so one M=128 matmul computes
    # 4 members' L0b at partition offsets 0/32/64/96).
    w0a_d = nc.dram_tensor("w0a", [S, KT, 128, E * 128], F32R, kind="ExternalInput")
    w0b_d = nc.dram_tensor("w0b4", [S, KT, 128, NQ * 128], F32R, kind="ExternalInput")
    w1a_d = nc.dram_tensor("w1a", [S, 128, E * H1], F32R, kind="ExternalInput")
    # w1b rows live at partition offset 32*(e%4) so the lhsT slice's base
    # partition matches the g0b-merged rhs slice (PE requires equal bases).
    w1b_d = nc.dram_tensor("w1b", [S, 128, E * H1], F32R, kind="ExternalInput")
    # w2 packed for member-merged L2: per quad, the four members' 96-row
    # outputs are packed into 3 psum banks of 128 partitions:
    #   bank0 = e0[0:96] | e1[0:32];  bank1 = e1[32:96] | e2[0:64];
    #   bank2 = e2[64:96] | e3[0:96]
    # Each bank takes 2 matmuls (one per contributing member) with
    # zero-padded lhsT column blocks.  Pack: [S, NQ, 3, 2, 128, 128]
    # -> flatten cols: [S, 128, NQ*3*2*128]
    w2_d = nc.dram_tensor("w2p", [S, 128, NQ * 6 * 128], F32R, kind="ExternalInput")
    # bias packs; *_a = b/alpha (ACT bias), *_d = b (DVE blend bias).
    # b0a: one column per (s,e).  b0b: one column per (s,quad) with the four
    # members' 32-row bias slices stacked.
    b0a_a = nc.dram_tensor("b0a_a", [128, S * E], F32, kind="ExternalInput")
    b0a_d = nc.dram_tensor("b0a_d", [128, S * E], F32, kind="ExternalInput")
    b0b_a = nc.dram_tensor("b0b_a", [128, S * NQ], F32, kind="ExternalInput")
    b0b_d = nc.dram_tensor("b0b_d", [128, S * NQ], F32, kind="ExternalInput")
    b1_a = nc.dram_tensor("b1_a", [H1, S * E], F32, kind="ExternalInput")
    b1_d = nc.dram_tensor("b1_d", [H1, S * E], F32, kind="ExternalInput")
    # merged-L2 bias packs: one column per (s, quad, bank)
    b2_a = nc.dram_tensor("b2m_a", [128, S * NQ * 3], F32, kind="ExternalInput")
    b2_d = nc.dram_tensor("b2m_d", [128, S * NQ * 3], F32, kind="ExternalInput")
    # output: per-(s,quad,bank,chunk) row-sums of g2 (merged-row layout)
    rs_d = nc.dram_tensor("rs", [128, S * NQ * 3 * NCH], F32, kind="ExternalOutput")

    with tile.TileContext(nc) as tc:
        with (
            tc.tile_pool(name="xp", bufs=2) as xp,
            tc.tile_pool(name="w0pool", bufs=2) as w0p,
            tc.tile_pool(name="w1pool", bufs=2) as w1p,
            tc.tile_pool(name="bp", bufs=1) as bp,
            tc.tile_pool(name="ep", bufs=6) as ep,
            tc.tile_pool(name="gp", bufs=6) as gp,
            tc.tile_pool(name="ps", bufs=2, space="PSUM") as psp,
        ):
            # warm the ACT Exp table during the initial DMA wait (the
            # PSEUDO_LOAD_ACT_FUNC_SET rides on the first ACTIVATE, ~2.7us)
            warm = bp.tile([1, 1], F32, tag="warm", name="warm")
            nc.vector.memset(warm[:], 0.0)
            nc.scalar.activation(warm[:], warm[:], EXP)

            # biases (emitted after the first species' critical DMAs below;
            # tiles declared here, loaded lazily)
            B = {}
            _bias_dmas = []
            for nm, d, p in (
                ("b0a_a", b0a_a, 128), ("b0a_d", b0a_d, 128),
                ("b0b_a", b0b_a, 128), ("b0b_d", b0b_d, 128),
                ("b1_a", b1_a, H1), ("b1_d", b1_d, H1),
                ("b2_a", b2_a, 128), ("b2_d", b2_d, 128),
            ):
                t = bp.tile([p, d.shape[-1]], F32, tag=nm, name=nm)
                _bias_dmas.append((t, d))
                B[nm] = t
            RS = bp.tile([128, S * NQ * 3 * NCH], F32, tag="RS", name="RS")

            for s in range(S):
                xk = []
                w0ak = []
                w0bk = []
                # first-chunk x slices + all weights first, so chunk-0 compute
                # starts as early as possible; remaining x chunks stream after
                for k in range(KT):
                    xt = xp.tile([128, A_SP], F32R, tag=f"x{k}", name=f"x_{s}_{k}")
                    nc.sync.dma_start(xt[:, 0:CHUNK], xt_d[s, k, :, 0:CHUNK])
                    xk.append(xt)
                for k in range(KT):
                    wt = w0p.tile([128, E * 128], F32R, tag=f"w0a{k}", name=f"w0a_{s}_{k}")
                    nc.sync.dma_start(wt[:], w0a_d[s, k])
                    w0ak.append(wt)
                    wbt = w0p.tile([128, NQ * 128], F32R, tag=f"w0b{k}", name=f"w0b_{s}_{k}")
                    nc.sync.dma_start(wbt[:], w0b_d[s, k])
                    w0bk.append(wbt)
                w1at = w1p.tile([128, E * H1], F32R, tag="w1a", name=f"w1a_{s}")
                nc.sync.dma_start(w1at[:], w1a_d[s])
                w1bt = w1p.tile([128, E * H1], F32R, tag="w1b", name=f"w1b_{s}")
                nc.sync.dma_start(w1bt[:], w1b_d[s])
                w2t = w1p.tile([128, NQ * 6 * 128], F32R, tag="w2", name=f"w2_{s}")
                nc.sync.dma_start(w2t[:], w2_d[s])
                if s == 0:
                    # bias packs: needed by the first ACT (~30us in), emitted
                    # after the critical first-chunk x + weight transfers
                    for t, d in _bias_dmas:
                        nc.sync.dma_start(t[:], d[:])
                for k in range(KT):
                    nc.sync.dma_start(
                        xk[k][:, CHUNK:A_SP], xt_d[s, k, :, CHUNK:A_SP]
                    )

                for c in range(NCH):
                    cs = slice(c * CHUNK, (c + 1) * CHUNK)
                    for q in range(NQ):
                        sq = s * NQ + q
                        # ---- merged layer-0b for the 4 members of this quad:
                        # one M=128 matmul per K-tile computes the four
                        # members' last-32 features at partition offsets
                        # 0/32/64/96 (they share rhs x).
                        ps0b = psp.tile([128, CHUNK], F32, tag="l0b", bufs=2)
                        for k in range(KT):
                            nc.tensor.matmul(
                                ps0b[:],
                                w0bk[k][:, q * 128 : (q + 1) * 128],
                                xk[k][:, cs],
                                start=(k == 0),
                                stop=(k == KT - 1),
                            )
                        e0b = ep.tile([128, CHUNK], F32, tag="e0b")
                        nc.scalar.activation(
                            e0b[:], ps0b[:], EXP,
                            bias=B["b0b_a"][:, sq : sq + 1], scale=1.0 / ALPHA,
                        )
                        g0b = gp.tile([128, CHUNK], F32R, tag="g0b")
                        nc.vector._custom_dve(
                            CELU, out=g0b[:], in0=ps0b[:], in1=e0b[:],
                            s0=B["b0b_d"][:, sq : sq + 1], s1=ALPHA,
                        )
                        def do_l2_bank(b):
                            # merged layer 2, bank b of the quad: the 4x96
                            # outputs pack into 3 psum banks, 2 zero-padded
                            # matmuls per bank; emitted as soon as both
                            # contributing members' g1 exist.
                            (m0, m1) = ((0, 1), (1, 2), (2, 3))[b]
                            ps2 = psp.tile([128, CHUNK], F32, tag="l2", name=f"ps2_{b}")
                            off = (q * 3 + b) * 2 * 128
                            nc.tensor.matmul(
                                ps2[:], w2t[:, off : off + 128], g1s[m0][:],
                                start=True, stop=False,
                            )
                            nc.tensor.matmul(
                                ps2[:], w2t[:, off + 128 : off + 256], g1s[m1][:],
                                start=False, stop=True,
                            )
                            sqb = (s * NQ + q) * 3 + b
                            e2 = ep.tile([128, CHUNK], F32, tag="e2", name=f"e2_{b}")
                            nc.scalar.activation(
                                e2[:], ps2[:], EXP,
                                bias=B["b2_a"][:, sqb : sqb + 1], scale=1.0 / ALPHA,
                            )
                            g2 = gp.tile([128, CHUNK], F32, tag="g2", name=f"g2_{b}")
                            nc.vector._custom_dve(
                                CELU_ACC, out=g2[:],
                                accum_out=RS[:, sqb * NCH + c : sqb * NCH + c + 1],
                                in0=ps2[:], in1=e2[:],
                                s0=B["b2_d"][:, sqb : sqb + 1], s1=ALPHA,
                            )

                        g1s = []
                        for e in range(q * 4, q * 4 + 4):
                            se = s * E + e
                            # ---- layer 0a (first 128 features of member e)
                            ps0a = psp.tile([128, CHUNK], F32, tag="l0a", bufs=2)
                            for k in range(KT):
                                nc.tensor.matmul(
                                    ps0a[:],
                                    w0ak[k][:, e * 128 : (e + 1) * 128],
                                    xk[k][:, cs],
                                    start=(k == 0),
                                    stop=(k == KT - 1),
                                )
                            e0a = ep.tile([128, CHUNK], F32, tag="e0a")
                            nc.scalar.activation(
                                e0a[:], ps0a[:], EXP,
                                bias=B["b0a_a"][:, se : se + 1], scale=1.0 / ALPHA,
                            )
                            g0a = gp.tile([128, CHUNK], F32R, tag="g0a")
                            nc.vector._custom_dve(
                                CELU, out=g0a[:], in0=ps0a[:], in1=e0a[:],
                                s0=B["b0a_d"][:, se : se + 1], s1=ALPHA,
                            )
                            # ---- layer 1
                            ps1 = psp.tile([H1, CHUNK], F32, tag="l1", bufs=2)
                            nc.tensor.matmul(
                                ps1[:], w1at[:, e * H1 : (e + 1) * H1], g0a[:],
                                start=True, stop=False,
                            )
                            # K=128 against the full merged g0b; w1bt has member
                            # e's 32 rows at offset 32*(e%4) and zeros elsewhere,
                            # so the other members' rows contribute nothing.
                            nc.tensor.matmul(
                                ps1[:], w1bt[:, e * H1 : (e + 1) * H1], g0b[:],
                                start=False, stop=True,
                            )
                            e1 = ep.tile([H1, CHUNK], F32, tag="e1")
                            nc.scalar.activation(
                                e1[:], ps1[:], EXP,
                                bias=B["b1_a"][:, se : se + 1], scale=1.0 / ALPHA,
                            )
                            g1 = gp.tile([H1, CHUNK], F32R, tag="g1", bufs=6)
                            nc.vector._custom_dve(
                                CELU, out=g1[:], in0=ps1[:], in1=e1[:],
                                s0=B["b1_d"][:, se : se + 1], s1=ALPHA,
                            )
                            g1s.append(g1)
                            # bank b of the merged L2 needs g1s[b] and g1s[b+1]
                            if len(g1s) >= 2:
                                do_l2_bank(len(g1s) - 2)
            nc.sync.dma_start(rs_d[:], RS[:])
    nc.compile()
    _NC = nc
    return nc


# ------------------------------------------------------------- host side
# merged-L2 bank layout: per quad, (bank, piece) -> (member_in_quad,
# w2-col range, psum-row offset)
_L2_PIECES = [
    [(0, 0, 96, 0), (1, 0, 32, 96)],
    [(1, 32, 96, 0), (2, 0, 64, 64)],
    [(2, 64, 96, 0), (3, 0, 96, 32)],
]


def _prep_shared(w0, w1, w2, b0, b1, b2):
    """Pack weights/biases into the device layouts (replicated to all cores)."""
    f = np.float32
    w0r = w0.reshape(S, E, KT, 128, H0)
    # w0a [S,KT,128,E*128]: [s,k,p, e*128+m] = w0[s,e,k*128+p,m], m<128
    w0a = np.ascontiguousarray(
        w0r[..., :128].transpose(0, 2, 3, 1, 4).reshape(S, KT, 128, E * 128)
    ).astype(f)
    # w0b4 [S,KT,128,NQ*128]: quad q's cols pack members 4q..4q+3's last-32
    # features: col q*128 + i*32 + m = w0[s,4q+i,k*128+p,128+m]
    w0b4 = np.ascontiguousarray(
        w0r[..., 128:].transpose(0, 2, 3, 1, 4).reshape(S, KT, 128, E * (H0 - 128))
    ).astype(f)
    w1a = np.ascontiguousarray(
        w1[:, :, :128, :].transpose(0, 2, 1, 3).reshape(S, 128, E * H1)
    ).astype(f)
    # w1b [S,128,E*H1], member e's 32 rows at partition offset 32*(e%4)
    w1b = np.zeros((S, 4, 32, E, H1), dtype=f)
    for e in range(E):
        w1b[:, e % 4, :, e, :] = w1[:, e, 128:, :]
    w1b = np.ascontiguousarray(w1b.reshape(S, 128, E * H1))
    # merged-L2 packs
    w2pk = np.zeros((S, NQ, 3, 2, 128, 128), dtype=f)  # [s,q,b,piece,K,M]
    b2m = np.zeros((S, NQ, 3, 128), dtype=f)
    for s in range(S):
        for q in range(NQ):
            for b in range(3):
                for piece, (mi, lo, hi, row) in enumerate(_L2_PIECES[b]):
                    e = 4 * q + mi
                    w2pk[s, q, b, piece, :, row : row + hi - lo] = w2[s, e, :, lo:hi]
                    b2m[s, q, b, row : row + hi - lo] = b2[s, e, 0, lo:hi]
    w2p = np.ascontiguousarray(
        w2pk.transpose(0, 4, 1, 2, 3, 5).reshape(S, 128, NQ * 6 * 128)
    )

    def bias_pack(b, lo, hi):
        # b [S,E,1,P] -> [hi-lo, S*E]
        return np.ascontiguousarray(b[:, :, 0, lo:hi].reshape(S * E, hi - lo).T).astype(f)

    # b0b pack [128, S*NQ]: col s*NQ+q rows i*32+m = b0[s,4q+i,0,128+m]
    b0b_pack = np.ascontiguousarray(
        b0[:, :, 0, 128:].reshape(S, NQ, 4 * (H0 - 128)).transpose(2, 0, 1).reshape(128, S * NQ)
    ).astype(f)
    b2m_pack = np.ascontiguousarray(
        b2m.reshape(S * NQ * 3, 128).T
    ).astype(f)

    shared = {
        "w0a": w0a, "w0b4": w0b4, "w1a": w1a, "w1b": w1b, "w2p": w2p,
        "b0a_d": bias_pack(b0, 0, 128), "b0b_d": b0b_pack,
        "b1_d": bias_pack(b1, 0, H1), "b2m_d": b2m_pack,
    }
    shared["b0a_a"] = (shared["b0a_d"] / ALPHA).astype(f)
    shared["b0b_a"] = (shared["b0b_d"] / ALPHA).astype(f)
    shared["b1_a"] = (shared["b1_d"] / ALPHA).astype(f)
    shared["b2m_a"] = (shared["b2m_d"] / ALPHA).astype(f)
    return shared


def _run(inputs, trace=False, tmpdir=None):
    aev = np.asarray(inputs["aev"], dtype=np.float32)
    idx = np.asarray(inputs["idx"], dtype=np.int32)
    w3 = np.asarray(inputs["w3"], dtype=np.float32)
    b3 = np.asarray(inputs["b3"], dtype=np.float32)

    nc = _build_nc()
    shared = _prep_shared(
        np.asarray(inputs["w0"], dtype=np.float32),
        np.asarray(inputs["w1"], dtype=np.float32),
        np.asarray(inputs["w2"], dtype=np.float32),
        np.asarray(inputs["b0"], dtype=np.float32),
        np.asarray(inputs["b1"], dtype=np.float32),
        np.asarray(inputs["b2"], dtype=np.float32),
    )

    aev_flat = aev.reshape(-1, K0)
    in_maps = []
    for c in range(N_CORES):
        idx_c = idx[:, c * A_SP : (c + 1) * A_SP]                # [S, A_SP]
        x = aev_flat[idx_c.reshape(-1)].reshape(S, A_SP, K0)     # [S, A_SP, 384]
        xt = np.ascontiguousarray(x.transpose(0, 2, 1)).reshape(S, KT, 128, A_SP)
        in_maps.append({"xt": xt, **shared})

    res = run_bass_kernel_spmd(
        nc, in_maps, core_ids=list(range(N_CORES)), trace=trace, tmpdir=tmpdir
    )

    # host-side tail: w3 dot (on the merged-row layout) + b3 + ensemble mean
    # + cross-core sum
    w3m = np.zeros((128, S, NQ, 3), dtype=np.float64)
    for s in range(S):
        for q in range(NQ):
            for b in range(3):
                for (mi, lo, hi, row) in _L2_PIECES[b]:
                    w3m[row : row + hi - lo, s, q, b] = w3[s, 4 * q + mi, lo:hi, 0]
    w3rep = np.repeat(
        w3m.reshape(128, S * NQ * 3)[:, :, None], NCH, axis=2
    ).reshape(128, S * NQ * 3 * NCH)
    total = 0.0
    for c in range(N_CORES):
        total += float((res.results[c]["rs"].astype(np.float64) * w3rep).sum())
    total += float(b3.astype(np.float64).sum()) * (N_ATOMS // S)
    out = np.array([total / E], dtype=np.float32)
    return out, res


def kernel(**inputs):
    out, _ = _run(inputs, trace=bool(int(os.environ.get("BASS_KERNEL_TRACE", "0"))))
    return out


# revision 37
# speedup vs baseline: 1.0107x; 1.0107x over previous
"""Trainium2 Bass kernel for nn_BmmEnsemble (ANI-style per-species ensemble MLP).

Math (see reference): for each species s (4) and ensemble member e (8), the
species' atoms' AEV rows go through a 384->160->128->96->1 MLP with CELU(0.1)
after the first three layers; the output energy is the global sum over all
atoms of the ensemble-mean of the final scalar, i.e.

    E = (1/8) * sum_{s,e,n} ( g2[s,e,n,:] @ w3[s,e] + b3[s,e] )

Distribution: data-parallel over atoms. The host gathers each species' atom
rows (aev_flat[idx]) and hands each of the 8 cores a 2048-atoms-per-species
slice, pre-transposed to feature-major [384, 2048] so activations stay
feature-major on-chip (no transposes between layers). Per-species ensemble
weights are replicated to all cores. Each core returns per-(s,e) row-sums of
the last hidden layer [96, 32*4]; the host applies the tiny w3 dot, the b3
term, the ensemble mean, and the cross-core sum (the "all-reduce").

On-chip design (all matmuls float32r = full-rate rounded fp32, feature-major
layout so no transposes are ever needed between layers; all activation tiles
are full 128-partition x 512-atom tiles):
  - Layer 0 per member: psum_l0a[128,512] += w0a[k].T @ xT[k] (3 K-tiles).
  - Layer 0 "tail" (features 128:160): the four members of a quad share rhs x,
    so their 32-feature blocks are column-packed into ONE M=128 matmul per
    K-tile, landing at partition offsets 0/32/64/96 of a shared psum bank.
  - Layer 1 per member: K=128 matmul on g0a plus a K=128 matmul against the
    full merged g0b with a zero-padded w1b (member's 32 rows at offset
    32*(e%4)) - zero rows contribute nothing, and K=128 costs the same as
    K=32 (matmul time is set by the moving dim).
  - Layer 2: the quad's 4x96 outputs are packed into 3 psum banks
    (bank0 = e0[0:96]|e1[0:32], bank1 = e1[32:96]|e2[0:64],
    bank2 = e2[64:96]|e3[0:96]) via 2 zero-padded matmuls per bank.
  - ScalarE: ONE Exp pass per psum tile: e = Exp((z_pre + b)/alpha).
  - VectorE: ONE fused custom-DVE pass per psum tile:
      g = relu(z_pre + b) + min(alpha*e - alpha, 0)  ==  celu(z, alpha) exact
    (for layer 2 the same op also emits accum_out = row-sums of g2).

CELU identity used: celu(z) = relu(z) + min(alpha*(exp(z/alpha) - 1), 0).

Measured on 8 axon-tunneled trn2 cores: ~292 us HW exec, rel err 1.6e-4
(f32r matmul rounding; fp32 everywhere else). Engines all ~85-90% busy
(VectorE 90%, ScalarE 87%, TensorE 84%) - the kernel is at the balanced
three-engine floor of this dataflow: the two PSUM-read elementwise passes
(exp on ScalarE, celu-blend on VectorE) stream at 1 elem/lane/cycle and
bound the kernel at ~205-270 us/core alongside 928 matmuls at ~253 us.
"""

import os
from operator import add as _operator_add

import numpy as np

import concourse.dve_ops as _dve_ops
import concourse.mybir as mybir
import concourse.tile as tile
from concourse import bacc
from concourse.bass_utils import run_bass_kernel_spmd
from concourse.dve_spec import (
    C0,
    C1,
    Spec,
    Src0,
    Src1,
    Zero,
    _has_src1,
    lower,
    minn,
    relu,
)
from concourse.dve_uop import DveOpSpec

# ---------------------------------------------------------------- constants
S, E = 4, 8
N_ATOMS = 65536
N_CORES = 8
A_SP = N_ATOMS // S // N_CORES      # atoms per species per core = 2048
CHUNK = 512
NCH = A_SP // CHUNK                 # 4 chunks
K0, H0, H1, H2 = 384, 160, 128, 96
KT = K0 // 128                      # 3 K-tiles for layer 0
NQ = 2                              # member quads per species (E/4)
ALPHA = 0.1

F32 = mybir.dt.float32
F32R = mybir.dt.float32r
EXP = mybir.ActivationFunctionType.Exp

# ------------------------------------------------------- custom DVE ops
_BODY = relu(Src0 + C0) + minn(Src1 * C1 - C1, Zero)


def _celu_np(in0, in1, s0, s1):
    z = in0.astype(np.float32) + s0
    neg = np.minimum(in1.astype(np.float32) * s1 - s1, 0.0)
    return (np.maximum(z, 0.0) + neg).astype(np.float32)


def _ref_plain(in0, in1, s0, s1, imm2):
    return _celu_np(in0, in1, s0, s1)


def _ref_acc(in0, in1, s0, s1, imm2):
    b = _celu_np(in0, in1, s0, s1)
    return b, b.reshape(b.shape[0], -1).sum(axis=-1, keepdims=True)


def _mk_op(name, spec):
    row = _dve_ops._CUSTOM_DVE_ROW_BASE + len(_dve_ops.OPS)
    assert row < 0x20, "custom-DVE opcode rows exhausted"
    _dve_ops._SUB_OPCODE_FOR_NAME[name] = row
    shas = {}
    for ver in ("v3", "v4"):
        s = DveOpSpec(
            name=name, opcode=row, uops=lower(spec, ver=ver), rd1_en=_has_src1(spec)
        )
        shas[ver] = s.sha(ver)
    op = _dve_ops.DveOp(name, spec, subdim=False, uops_sha=shas)
    _dve_ops.OPS.append(op)
    _dve_ops.CUSTOM_DVE_SPECS[name] = spec
    return op


def _register_celu_ops():
    existing = {o.name: o for o in _dve_ops.OPS}
    if "CELU_BLEND_ANT" in existing:
        return existing["CELU_BLEND_ANT"], existing["CELU_BLEND_ACC_ANT"]
    celu = _mk_op("CELU_BLEND_ANT", Spec(body=_BODY, reference=_ref_plain))
    celu_acc = _mk_op(
        "CELU_BLEND_ACC_ANT",
        Spec(body=_BODY, accum=_operator_add, accum_init=Zero, reference=_ref_acc),
    )
    return celu, celu_acc


# ------------------------------------------------------------ device build
_NC = None


def _build_nc():
    global _NC
    if _NC is not None:
        return _NC
    CELU, CELU_ACC = _register_celu_ops()

    nc = bacc.Bacc("TRN2", target_bir_lowering=False, debug=False)

    # per-core inputs
    xt_d = nc.dram_tensor("xt", [S, KT, 128, A_SP], F32R, kind="ExternalInput")
    # replicated weight packs.
    # w0a: first 128 output features per member.  w0b4: the last 32 features
    # of FOUR members column-packed into one [*,128] stationary tile (all
    # members of a species share the same x, so one M=128 matmul computes
    # 4 members' L0b at partition offsets 0/32/64/96).
    w0a_d = nc.dram_tensor("w0a", [S, KT, 128, E * 128], F32R, kind="ExternalInput")
    w0b_d = nc.dram_tensor("w0b4", [S, KT, 128, NQ * 128], F32R, kind="ExternalInput")
    w1a_d = nc.dram_tensor("w1a", [S, 128, E * H1], F32R, kind="ExternalInput")
    # w1b rows live at partition offset 32*(e%4) so the lhsT slice's base
    # partition matches the g0b-merged rhs slice (PE requires equal bases).
    w1b_d = nc.dram_tensor("w1b", [S, 128, E * H1], F32R, kind="ExternalInput")
    # w2 packed for member-merged L2: per quad, the four members' 96-row
    # outputs are packed into 3 psum banks of 128 partitions:
    #   bank0 = e0[0:96] | e1[0:32];  bank1 = e1[32:96] | e2[0:64];
    #   bank2 = e2[64:96] | e3[0:96]
    # Each bank takes 2 matmuls (one per contributing member) with
    # zero-padded lhsT column blocks.  Pack: [S, NQ, 3, 2, 128, 128]
    # -> flatten cols: [S, 128, NQ*3*2*128]
    w2_d = nc.dram_tensor("w2p", [S, 128, NQ * 6 * 128], F32R, kind="ExternalInput")
    # bias packs; *_a = b/alpha (ACT bias), *_d = b (DVE blend bias).
    # b0a: one column per (s,e).  b0b: one column per (s,quad) with the four
    # members' 32-row bias slices stacked.
    b0a_a = nc.dram_tensor("b0a_a", [128, S * E], F32, kind="ExternalInput")
    b0a_d = nc.dram_tensor("b0a_d", [128, S * E], F32, kind="ExternalInput")
    b0b_a = nc.dram_tensor("b0b_a", [128, S * NQ], F32, kind="ExternalInput")
    b0b_d = nc.dram_tensor("b0b_d", [128, S * NQ], F32, kind="ExternalInput")
    b1_a = nc.dram_tensor("b1_a", [H1, S * E], F32, kind="ExternalInput")
    b1_d = nc.dram_tensor("b1_d", [H1, S * E], F32, kind="ExternalInput")
    # merged-L2 bias packs: one column per (s, quad, bank)
    b2_a = nc.dram_tensor("b2m_a", [128, S * NQ * 3], F32, kind="ExternalInput")
    b2_d = nc.dram_tensor("b2m_d", [128, S * NQ * 3], F32, kind="ExternalInput")
    # output: per-(s,quad,bank,chunk) row-sums of g2 (merged-row layout)
    rs_d = nc.dram_tensor("rs", [128, S * NQ * 3 * NCH], F32, kind="ExternalOutput")

    with tile.TileContext(nc) as tc:
        with (
            tc.tile_pool(name="xp", bufs=2) as xp,
            tc.tile_pool(name="w0pool", bufs=2) as w0p,
            tc.tile_pool(name="w1pool", bufs=2) as w1p,
            tc.tile_pool(name="bp", bufs=1) as bp,
            tc.tile_pool(name="ep", bufs=6) as ep,
            tc.tile_pool(name="gp", bufs=6) as gp,
            tc.tile_pool(name="ps", bufs=2, space="PSUM") as psp,
        ):
            # warm the ACT Exp table during the initial DMA wait (the
            # PSEUDO_LOAD_ACT_FUNC_SET rides on the first ACTIVATE, ~2.7us)
            warm = bp.tile([1, 1], F32, tag="warm", name="warm")
            nc.vector.memset(warm[:], 0.0)
            nc.scalar.activation(warm[:], warm[:], EXP)

            # biases (emitted after the first species' critical DMAs below;
            # tiles declared here, loaded lazily)
            B = {}
            _bias_dmas = []
            for nm, d, p in (
                ("b0a_a", b0a_a, 128), ("b0a_d", b0a_d, 128),
                ("b0b_a", b0b_a, 128), ("b0b_d", b0b_d, 128),
                ("b1_a", b1_a, H1), ("b1_d", b1_d, H1),
                ("b2_a", b2_a, 128), ("b2_d", b2_d, 128),
            ):
                t = bp.tile([p, d.shape[-1]], F32, tag=nm, name=nm)
                _bias_dmas.append((t, d))
                B[nm] = t
            RS = bp.tile([128, S * NQ * 3 * NCH], F32, tag="RS", name="RS")

            for s in range(S):
                xk = []
                w0ak = []
                w0bk = []
                # first-chunk x slices + all weights first, so chunk-0 compute
                # starts as early as possible; remaining x chunks stream after
                for k in range(KT):
                    xt = xp.tile([128, A_SP], F32R, tag=f"x{k}", name=f"x_{s}_{k}")
                    nc.sync.dma_start(xt[:, 0:CHUNK], xt_d[s, k, :, 0:CHUNK])
                    xk.append(xt)
                for k in range(KT):
                    wt = w0p.tile([128, E * 128], F32R, tag=f"w0a{k}", name=f"w0a_{s}_{k}")
                    nc.sync.dma_start(wt[:], w0a_d[s, k])
                    w0ak.append(wt)
                    wbt = w0p.tile([128, NQ * 128], F32R, tag=f"w0b{k}", name=f"w0b_{s}_{k}")
                    nc.sync.dma_start(wbt[:], w0b_d[s, k])
                    w0bk.append(wbt)
                w1at = w1p.tile([128, E * H1], F32R, tag="w1a", name=f"w1a_{s}")
                nc.sync.dma_start(w1at[:], w1a_d[s])
                w1bt = w1p.tile([128, E * H1], F32R, tag="w1b", name=f"w1b_{s}")
                nc.sync.dma_start(w1bt[:], w1b_d[s])
                w2t = w1p.tile([128, NQ * 6 * 128], F32R, tag="w2", name=f"w2_{s}")
                nc.sync.dma_start(w2t[:], w2_d[s])
                if s == 0:
                    # bias packs: needed by the first ACT (~30us in), emitted
                    # after the critical first-chunk x + weight transfers
                    for t, d in _bias_dmas:
                        nc.sync.dma_start(t[:], d[:])
                for k in range(KT):
                    nc.sync.dma_start(
                        xk[k][:, CHUNK:A_SP], xt_d[s, k, :, CHUNK:A_SP]
                    )

                for c in range(NCH):
                    cs = slice(c * CHUNK, (c + 1) * CHUNK)
                    for q in range(NQ):
                        sq = s * NQ + q
                        # ---- merged layer-0b for the 4 members of this quad:
                        # one M=128 matmul per K-tile computes the four
                        # members' last-32 features at partition offsets
                        # 0/32/64/96 (they share rhs x).
                        ps0b = psp.tile([128, CHUNK], F32, tag="l0b", bufs=1)
                        for k in range(KT):
                            nc.tensor.matmul(
                                ps0b[:],
                                w0bk[k][:, q * 128 : (q + 1) * 128],
                                xk[k][:, cs],
                                start=(k == 0),
                                stop=(k == KT - 1),
                            )
                        e0b = ep.tile([128, CHUNK], F32, tag="e0b")
                        nc.scalar.activation(
                            e0b[:], ps0b[:], EXP,
                            bias=B["b0b_a"][:, sq : sq + 1], scale=1.0 / ALPHA,
                        )
                        g0b = gp.tile([128, CHUNK], F32R, tag="g0b")
                        nc.vector._custom_dve(
                            CELU, out=g0b[:], in0=ps0b[:], in1=e0b[:],
                            s0=B["b0b_d"][:, sq : sq + 1], s1=ALPHA,
                        )
                        def do_l2_bank(b):
                            # merged layer 2, bank b of the quad: the 4x96
                            # outputs pack into 3 psum banks, 2 zero-padded
                            # matmuls per bank; emitted as soon as both
                            # contributing members' g1 exist.
                            (m0, m1) = ((0, 1), (1, 2), (2, 3))[b]
                            ps2 = psp.tile([128, CHUNK], F32, tag="l2", name=f"ps2_{b}")
                            off = (q * 3 + b) * 2 * 128
                            nc.tensor.matmul(
                                ps2[:], w2t[:, off : off + 128], g1s[m0][:],
                                start=True, stop=False,
                            )
                            nc.tensor.matmul(
                                ps2[:], w2t[:, off + 128 : off + 256], g1s[m1][:],
                                start=False, stop=True,
                            )
                            sqb = (s * NQ + q) * 3 + b
                            e2 = ep.tile([128, CHUNK], F32, tag="e2", name=f"e2_{b}")
                            nc.scalar.activation(
                                e2[:], ps2[:], EXP,
                                bias=B["b2_a"][:, sqb : sqb + 1], scale=1.0 / ALPHA,
                            )
                            g2 = gp.tile([128, CHUNK], F32, tag="g2", name=f"g2_{b}")
                            nc.vector._custom_dve(
                                CELU_ACC, out=g2[:],
                                accum_out=RS[:, sqb * NCH + c : sqb * NCH + c + 1],
                                in0=ps2[:], in1=e2[:],
                                s0=B["b2_d"][:, sqb : sqb + 1], s1=ALPHA,
                            )

                        g1s = []
                        for e in range(q * 4, q * 4 + 4):
                            se = s * E + e
                            # ---- layer 0a (first 128 features of member e)
                            ps0a = psp.tile([128, CHUNK], F32, tag="l0a", bufs=3)
                            for k in range(KT):
                                nc.tensor.matmul(
                                    ps0a[:],
                                    w0ak[k][:, e * 128 : (e + 1) * 128],
                                    xk[k][:, cs],
                                    start=(k == 0),
                                    stop=(k == KT - 1),
                                )
                            e0a = ep.tile([128, CHUNK], F32, tag="e0a")
                            nc.scalar.activation(
                                e0a[:], ps0a[:], EXP,
                                bias=B["b0a_a"][:, se : se + 1], scale=1.0 / ALPHA,
                            )
                            g0a = gp.tile([128, CHUNK], F32R, tag="g0a")
                            nc.vector._custom_dve(
                                CELU, out=g0a[:], in0=ps0a[:], in1=e0a[:],
                                s0=B["b0a_d"][:, se : se + 1], s1=ALPHA,
                            )
                            # ---- layer 1
                            ps1 = psp.tile([H1, CHUNK], F32, tag="l1", bufs=2)
                            nc.tensor.matmul(
                                ps1[:], w1at[:, e * H1 : (e + 1) * H1], g0a[:],
                                start=True, stop=False,
                            )
                            # K=128 against the full merged g0b; w1bt has member
                            # e's 32 rows at offset 32*(e%4) and zeros elsewhere,
                            # so the other members' rows contribute nothing.
                            nc.tensor.matmul(
                                ps1[:], w1bt[:, e * H1 : (e + 1) * H1], g0b[:],
                                start=False, stop=True,
                            )
                            e1 = ep.tile([H1, CHUNK], F32, tag="e1")
                            nc.scalar.activation(
                                e1[:], ps1[:], EXP,
                                bias=B["b1_a"][:, se : se + 1], scale=1.0 / ALPHA,
                            )
                            g1 = gp.tile([H1, CHUNK], F32R, tag="g1", bufs=6)
                            nc.vector._custom_dve(
                                CELU, out=g1[:], in0=ps1[:], in1=e1[:],
                                s0=B["b1_d"][:, se : se + 1], s1=ALPHA,
                            )
                            g1s.append(g1)
                            # bank b of the merged L2 needs g1s[b] and g1s[b+1]
                            if len(g1s) >= 2:
                                do_l2_bank(len(g1s) - 2)
            nc.sync.dma_start(rs_d[:], RS[:])
    nc.compile()
    _NC = nc
    return nc


# ------------------------------------------------------------- host side
# merged-L2 bank layout: per quad, (bank, piece) -> (member_in_quad,
# w2-col range, psum-row offset)
_L2_PIECES = [
    [(0, 0, 96, 0), (1, 0, 32, 96)],
    [(1, 32, 96, 0), (2, 0, 64, 64)],
    [(2, 64, 96, 0), (3, 0, 96, 32)],
]


def _prep_shared(w0, w1, w2, b0, b1, b2):
    """Pack weights/biases into the device layouts (replicated to all cores)."""
    f = np.float32
    w0r = w0.reshape(S, E, KT, 128, H0)
    # w0a [S,KT,128,E*128]: [s,k,p, e*128+m] = w0[s,e,k*128+p,m], m<128
    w0a = np.ascontiguousarray(
        w0r[..., :128].transpose(0, 2, 3, 1, 4).reshape(S, KT, 128, E * 128)
    ).astype(f)
    # w0b4 [S,KT,128,NQ*128]: quad q's cols pack members 4q..4q+3's last-32
    # features: col q*128 + i*32 + m = w0[s,4q+i,k*128+p,128+m]
    w0b4 = np.ascontiguousarray(
        w0r[..., 128:].transpose(0, 2, 3, 1, 4).reshape(S, KT, 128, E * (H0 - 128))
    ).astype(f)
    w1a = np.ascontiguousarray(
        w1[:, :, :128, :].transpose(0, 2, 1, 3).reshape(S, 128, E * H1)
    ).astype(f)
    # w1b [S,128,E*H1], member e's 32 rows at partition offset 32*(e%4)
    w1b = np.zeros((S, 4, 32, E, H1), dtype=f)
    for e in range(E):
        w1b[:, e % 4, :, e, :] = w1[:, e, 128:, :]
    w1b = np.ascontiguousarray(w1b.reshape(S, 128, E * H1))
    # merged-L2 packs
    w2pk = np.zeros((S, NQ, 3, 2, 128, 128), dtype=f)  # [s,q,b,piece,K,M]
    b2m = np.zeros((S, NQ, 3, 128), dtype=f)
    for s in range(S):
        for q in range(NQ):
            for b in range(3):
                for piece, (mi, lo, hi, row) in enumerate(_L2_PIECES[b]):
                    e = 4 * q + mi
                    w2pk[s, q, b, piece, :, row : row + hi - lo] = w2[s, e, :, lo:hi]
                    b2m[s, q, b, row : row + hi - lo] = b2[s, e, 0, lo:hi]
    w2p = np.ascontiguousarray(
        w2pk.transpose(0, 4, 1, 2, 3, 5).reshape(S, 128, NQ * 6 * 128)
    )

    def bias_pack(b, lo, hi):
        # b [S,E,1,P] -> [hi-lo, S*E]
        return np.ascontiguousarray(b[:, :, 0, lo:hi].reshape(S * E, hi - lo).T).astype(f)

    # b0b pack [128, S*NQ]: col s*NQ+q rows i*32+m = b0[s,4q+i,0,128+m]
    b0b_pack = np.ascontiguousarray(
        b0[:, :, 0, 128:].reshape(S, NQ, 4 * (H0 - 128)).transpose(2, 0, 1).reshape(128, S * NQ)
    ).astype(f)
    b2m_pack = np.ascontiguousarray(
        b2m.reshape(S * NQ * 3, 128).T
    ).astype(f)

    shared = {
        "w0a": w0a, "w0b4": w0b4, "w1a": w1a, "w1b": w1b, "w2p": w2p,
        "b0a_d": bias_pack(b0, 0, 128), "b0b_d": b0b_pack,
        "b1_d": bias_pack(b1, 0, H1), "b2m_d": b2m_pack,
    }
    shared["b0a_a"] = (shared["b0a_d"] / ALPHA).astype(f)
    shared["b0b_a"] = (shared["b0b_d"] / ALPHA).astype(f)
    shared["b1_a"] = (shared["b1_d"] / ALPHA).astype(f)
    shared["b2m_a"] = (shared["b2m_d"] / ALPHA).astype(f)
    return shared


def _run(inputs, trace=False, tmpdir=None):
    aev = np.asarray(inputs["aev"], dtype=np.float32)
    idx = np.asarray(inputs["idx"], dtype=np.int32)
    w3 = np.asarray(inputs["w3"], dtype=np.float32)
    b3 = np.asarray(inputs["b3"], dtype=np.float32)

    nc = _build_nc()
    shared = _prep_shared(
        np.asarray(inputs["w0"], dtype=np.float32),
        np.asarray(inputs["w1"], dtype=np.float32),
        np.asarray(inputs["w2"], dtype=np.float32),
        np.asarray(inputs["b0"], dtype=np.float32),
        np.asarray(inputs["b1"], dtype=np.float32),
        np.asarray(inputs["b2"], dtype=np.float32),
    )

    aev_flat = aev.reshape(-1, K0)
    in_maps = []
    for c in range(N_CORES):
        idx_c = idx[:, c * A_SP : (c + 1) * A_SP]                # [S, A_SP]
        x = aev_flat[idx_c.reshape(-1)].reshape(S, A_SP, K0)     # [S, A_SP, 384]
        xt = np.ascontiguousarray(x.transpose(0, 2, 1)).reshape(S, KT, 128, A_SP)
        in_maps.append({"xt": xt, **shared})

    res = run_bass_kernel_spmd(
        nc, in_maps, core_ids=list(range(N_CORES)), trace=trace, tmpdir=tmpdir
    )

    # host-side tail: w3 dot (on the merged-row layout) + b3 + ensemble mean
    # + cross-core sum
    w3m = np.zeros((128, S, NQ, 3), dtype=np.float64)
    for s in range(S):
        for q in range(NQ):
            for b in range(3):
                for (mi, lo, hi, row) in _L2_PIECES[b]:
                    w3m[row : row + hi - lo, s, q, b] = w3[s, 4 * q + mi, lo:hi, 0]
    w3rep = np.repeat(
        w3m.reshape(128, S * NQ * 3)[:, :, None], NCH, axis=2
    ).reshape(128, S * NQ * 3 * NCH)
    total = 0.0
    for c in range(N_CORES):
        total += float((res.results[c]["rs"].astype(np.float64) * w3rep).sum())
    total += float(b3.astype(np.float64).sum()) * (N_ATOMS // S)
    out = np.array([total / E], dtype=np.float32)
    return out, res


def kernel(**inputs):
    out, _ = _run(inputs, trace=bool(int(os.environ.get("BASS_KERNEL_TRACE", "0"))))
    return out
